# revision 13
# baseline (speedup 1.0000x reference)
"""Trainium2 Bass kernel for nn_AttentionBlock (retrieval_knn).

Data-parallel over batch across 8 cores. Per core (64 batches):
  Phase A: stream p (64MB) + x (8MB) + W1s (32MB) over the contraction dim
           (k=32768). PE-transpose 128x128 tiles of p/x to get k on
           partitions, then accumulate H1.T = W1.T @ p.T in PSUM
           (features-on-partitions layout).
  Mid:     MLP layers 2-4 (feature-major, no transposes), L2 norms via
           ones-matmul partition reduction, cross-dot scores, sharp softmax,
           top-2 candidate selection (softmax at sharpness 65536 is ~one-hot;
           residual weight mass < 1e-7 is dropped).
  Phase B: indirect-DMA gather of only the top-2 candidate patches in
           channel-major layout, then one matmul per batch folds the softmax
           weighting + fused g/o 1x1 conv (A = o_w@g_w) + gate: the gathered
           top-2 patches are stacked on 128 partitions and multiplied by
           [w1*sw*A.T ; w2*sw*A.T]. Gated blend with x finishes on DVE/ACT.
"""

import sys

import numpy as np

sys.path.insert(0, "/opt/trn_rl_repo")

import concourse.bass as bass
import concourse.bacc as bacc
import concourse.mybir as mybir
import concourse.tile as tile
from concourse.bass import IndirectOffsetOnAxis
from concourse.bass_utils import run_bass_kernel_spmd

AF = mybir.ActivationFunctionType
ALU = mybir.AluOpType
DT = mybir.dt

NCORES = 8
B, K, C, E = 512, 8, 64, 8
S = E * E * E            # 512
NIN = C * S              # 32768
CF, HID = 32, 128
BL = B // NCORES         # 64 batches per core
M = BL * K               # 512 p-rows per core
KC = 128                 # contraction chunk (partition dim)
NKC = NIN // KC          # 256 chunks
KBLK = 16                # chunks per DMA K-block
NKB = NKC // KBLK        # 16 K-blocks
SHARP = float(CF * S * 4)  # 65536
NB = 8                   # phase-B batch block size
NBLK = BL // NB          # 8 blocks

F32 = DT.float32
TRACE = False
TRACE_DIR = "/root/problem/trace_out"


def _leaky(nc, pool, h_psum, bias_ap, p, m, tagpfx):
    """a = leakyrelu(h + bias, 0.01); h in PSUM [p, m], bias [p, 1]."""
    y = pool.tile([p, m], F32, name=f"{tagpfx}_y", tag=f"{tagpfx}_y")
    nc.scalar.activation(y, h_psum, AF.Identity, bias=bias_ap, scale=1.0)
    t = pool.tile([p, m], F32, name=f"{tagpfx}_t", tag=f"{tagpfx}_t")
    nc.vector.tensor_scalar_mul(t, y, 0.01)
    a = pool.tile([p, m], F32, name=f"{tagpfx}_a", tag=f"{tagpfx}_a")
    nc.vector.tensor_tensor(out=a, in0=y, in1=t, op=ALU.max)
    return a


def build_nc():
    nc = bacc.Bacc("TRN2", target_bir_lowering=False, debug=False,
                   num_devices=NCORES)

    # ---- DRAM I/O ----
    xs = nc.dram_tensor("xs", [BL, NIN], F32, kind="ExternalInput").ap()
    ps = nc.dram_tensor("ps", [M, NIN], F32, kind="ExternalInput").ap()
    w1f = nc.dram_tensor("w1f", [NIN, HID], F32, kind="ExternalInput").ap()
    w1t = nc.dram_tensor("w1t", [NIN, HID], F32, kind="ExternalInput").ap()
    w2f = nc.dram_tensor("w2f", [HID, HID], F32, kind="ExternalInput").ap()
    w3f = nc.dram_tensor("w3f", [HID, HID], F32, kind="ExternalInput").ap()
    w4f = nc.dram_tensor("w4f", [HID, CF], F32, kind="ExternalInput").ap()
    w2t = nc.dram_tensor("w2t", [HID, HID], F32, kind="ExternalInput").ap()
    w3t = nc.dram_tensor("w3t", [HID, HID], F32, kind="ExternalInput").ap()
    w4t = nc.dram_tensor("w4t", [HID, CF], F32, kind="ExternalInput").ap()
    ident_d = nc.dram_tensor("ident", [128, 128], F32, kind="ExternalInput").ap()
    # consts columns: 0 b1t | 1 b2t | 2 b3t | 3 b4t(32) | 4 b1f | 5 b2f | 6 b3f
    # | 7 b4f(32) | 8 iota_c(64) | 9 ones | 10 8*iota_b(64) | 11..15 zero
    consts_d = nc.dram_tensor("consts", [128, 16], F32, kind="ExternalInput").ap()
    # crow row vector: cols 0:64 bf (= o_w@g_b + o_b), 64:128 ones
    crow_d = nc.dram_tensor("crow", [1, 128], F32, kind="ExternalInput").ap()
    bd_d = nc.dram_tensor("bdmask", [BL, M], F32, kind="ExternalInput").ap()
    iotak_d = nc.dram_tensor("iotak", [BL, K], F32, kind="ExternalInput").ap()
    atp2_d = nc.dram_tensor("atp2", [128, C], F32, kind="ExternalInput").ap()
    out_d = nc.dram_tensor("out", [BL, NIN], F32, kind="ExternalOutput").ap()
    dbg_d = nc.dram_tensor("dbg", [128, 64], F32, kind="ExternalOutput").ap()

    with tile.TileContext(nc) as tc:
        _build(nc, tc, xs, ps, w1f, w1t, w2f, w3f, w4f, w2t, w3t, w4t,
               ident_d, consts_d, crow_d, bd_d, iotak_d, atp2_d, out_d, dbg_d)
    nc.compile()
    return nc


def _build(nc, tc, xs, ps, w1f, w1t, w2f, w3f, w4f, w2t, w3t, w4t,
           ident_d, consts_d, crow_d, bd_d, iotak_d, atp2_d, out_d, dbg_d):
    from contextlib import ExitStack
    ctx = ExitStack()
    with ctx:
        consts = ctx.enter_context(tc.tile_pool(name="consts", bufs=1))
        # ---- resident constants ----
        ident = consts.tile([128, 128], F32, name="identsb")
        nc.sync.dma_start(ident, ident_d)
        cst = consts.tile([128, 16], F32, name="cstsb")
        nc.sync.dma_start(cst, consts_d)
        crow = consts.tile([1, 128], F32, name="crowsb")
        nc.sync.dma_start(crow, crow_d)
        bd = consts.tile([BL, M], F32, name="bdsb")
        nc.sync.dma_start(bd, bd_d)
        iotak = consts.tile([BL, K], F32, name="iotaksb")
        nc.sync.dma_start(iotak, iotak_d)
        atp2 = consts.tile([128, C], F32, name="atp2sb")
        nc.sync.dma_start(atp2, atp2_d)
        w2fs = consts.tile([HID, HID], F32, name="w2fs")
        nc.sync.dma_start(w2fs, w2f)
        w3fs = consts.tile([HID, HID], F32, name="w3fs")
        nc.sync.dma_start(w3fs, w3f)
        w4fs = consts.tile([HID, CF], F32, name="w4fs")
        nc.sync.dma_start(w4fs, w4f)
        w2ts = consts.tile([HID, HID], F32, name="w2ts")
        nc.sync.dma_start(w2ts, w2t)
        w3ts = consts.tile([HID, HID], F32, name="w3ts")
        nc.sync.dma_start(w3ts, w3t)
        w4ts = consts.tile([HID, CF], F32, name="w4ts")
        nc.sync.dma_start(w4ts, w4t)

        b1t, b2t, b3t = (cst[:, i:i + 1] for i in range(3))
        b4t = cst[:CF, 3:4]
        b1f, b2f, b3f = (cst[:, i:i + 1] for i in range(4, 7))
        b4f = cst[:CF, 7:8]
        iota_c = cst[:C, 8:9]
        iota_c2 = cst[:, 8:9]
        ones32 = cst[:CF, 9:10]
        one11 = cst[:1, 9:10]
        iotab8 = cst[:BL, 10:11]
        bf_row = crow[:, 0:C]
        ones_row = crow[:, C:C + BL]

        # persistent accumulators for H1 (whole phase A)
        pacc_cm = tc.tile_pool(name="pacc", bufs=1, space="PSUM")
        pacc = pacc_cm.__enter__()
        h1p = pacc.tile([128, M], F32, name="h1p")
        h1x = pacc.tile([128, BL], F32, name="h1x")

        # =========================== Phase A ===========================
        with tc.tile_pool(name="pin", bufs=2) as pin, \
             tc.tile_pool(name="prhs", bufs=4) as prhs, \
             tc.tile_pool(name="ptp", bufs=2, space="PSUM") as ptp, \
             tc.tile_pool(name="ptx", bufs=2, space="PSUM") as ptx:
            ps_v = ps.rearrange("(mb mp) n -> mp mb n", mp=128)      # [128,4,NIN]
            w1f_v = w1f.rearrange("(c k) n -> k c n", k=KC)          # [128,256,128]
            w1t_v = w1t.rearrange("(c k) n -> k c n", k=KC)
            for kb in range(NKB):
                k0 = kb * KBLK * KC
                p_nat = pin.tile([128, 4, KBLK * KC], F32, name="p_nat", tag="p_nat")
                nc.sync.dma_start(p_nat, ps_v[:, :, k0:k0 + KBLK * KC])
                x_nat = pin.tile([BL, KBLK * KC], F32, name="x_nat", tag="x_nat")
                nc.sync.dma_start(x_nat, xs[:, k0:k0 + KBLK * KC])
                w1f_b = pin.tile([128, KBLK, HID], F32, name="w1f_b", tag="w1f_b")
                nc.sync.dma_start(w1f_b, w1f_v[:, kb * KBLK:(kb + 1) * KBLK, :])
                w1t_b = pin.tile([128, KBLK, HID], F32, name="w1t_b", tag="w1t_b")
                nc.sync.dma_start(w1t_b, w1t_v[:, kb * KBLK:(kb + 1) * KBLK, :])
                for sc in range(KBLK):
                    g = kb * KBLK + sc
                    off = sc * KC
                    tp = ptp.tile([128, M], F32, name="tp", tag="tp")
                    for mb in range(4):
                        nc.tensor.transpose(
                            tp[:, mb * 128:(mb + 1) * 128],
                            p_nat[:, mb, off:off + KC], ident)
                    tx = ptx.tile([128, BL], F32, name="tx", tag="tx")
                    nc.tensor.transpose(tx, x_nat[:, off:off + KC],
                                        ident[:BL, :BL])
                    rhs = prhs.tile([128, M + BL], F32, name="rhs", tag="rhs")
                    nc.vector.tensor_copy(rhs[:, 0:M], tp)
                    nc.scalar.copy(rhs[:, M:M + BL], tx)
                    nc.tensor.matmul(h1p, lhsT=w1f_b[:, sc, :], rhs=rhs[:, 0:M],
                                     start=(g == 0), stop=(g == NKC - 1))
                    nc.tensor.matmul(h1x, lhsT=w1t_b[:, sc, :],
                                     rhs=rhs[:, M:M + BL],
                                     start=(g == 0), stop=(g == NKC - 1))

        # =========================== Mid phase =========================
        mid = ctx.enter_context(tc.tile_pool(name="mid", bufs=1))
        pmid_cm = tc.tile_pool(name="pmid", bufs=5, space="PSUM")
        pmid = pmid_cm.__enter__()

        # MLP layers 2..4, p path (feature-major: [feat, m])
        a1p = _leaky(nc, mid, h1p, b1f, 128, M, "a1p")
        h2p = pmid.tile([128, M], F32, name="h2p", tag="pm")
        nc.tensor.matmul(h2p, lhsT=w2fs, rhs=a1p, start=True, stop=True)
        a2p = _leaky(nc, mid, h2p, b2f, 128, M, "a2p")
        h3p = pmid.tile([128, M], F32, name="h3p", tag="pm")
        nc.tensor.matmul(h3p, lhsT=w3fs, rhs=a2p, start=True, stop=True)
        a3p = _leaky(nc, mid, h3p, b3f, 128, M, "a3p")
        h4p = pmid.tile([CF, M], F32, name="h4p", tag="pm")
        nc.tensor.matmul(h4p, lhsT=w4fs, rhs=a3p, start=True, stop=True)
        # x path
        a1x = _leaky(nc, mid, h1x, b1t, 128, BL, "a1x")
        h2x = pmid.tile([128, BL], F32, name="h2x", tag="pm")
        nc.tensor.matmul(h2x, lhsT=w2ts, rhs=a1x, start=True, stop=True)
        a2x = _leaky(nc, mid, h2x, b2t, 128, BL, "a2x")
        h3x = pmid.tile([128, BL], F32, name="h3x", tag="pm")
        nc.tensor.matmul(h3x, lhsT=w3ts, rhs=a2x, start=True, stop=True)
        a3x = _leaky(nc, mid, h3x, b3t, 128, BL, "a3x")
        h4x = pmid.tile([CF, BL], F32, name="h4x", tag="pm")
        nc.tensor.matmul(h4x, lhsT=w4ts, rhs=a3x, start=True, stop=True)

        # features + squares (bias folded in)
        fp = mid.tile([CF, M], F32, name="fp")
        nc.scalar.activation(fp, h4p, AF.Identity, bias=b4f, scale=1.0)
        sqp = mid.tile([CF, M], F32, name="sqp")
        nc.scalar.activation(sqp, h4p, AF.Square, bias=b4f, scale=1.0)
        fx = mid.tile([CF, BL], F32, name="fx")
        nc.scalar.activation(fx, h4x, AF.Identity, bias=b4t, scale=1.0)
        sqx = mid.tile([CF, BL], F32, name="sqx")
        nc.scalar.activation(sqx, h4x, AF.Square, bias=b4t, scale=1.0)

        # L2 norms: partition-reduce via ones matmul
        n2p = pmid.tile([1, M], F32, name="n2p", tag="pm")
        nc.tensor.matmul(n2p, lhsT=ones32, rhs=sqp, start=True, stop=True)
        n2x = pmid.tile([1, BL], F32, name="n2x", tag="pm")
        nc.tensor.matmul(n2x, lhsT=ones32, rhs=sqx, start=True, stop=True)
        nrp = mid.tile([1, M], F32, name="nrp")
        nc.scalar.activation(nrp, n2p, AF.Sqrt)
        nc.vector.tensor_scalar_max(nrp, nrp, 1e-12)
        rnp = mid.tile([1, M], F32, name="rnp")
        nc.vector.reciprocal(rnp, nrp)
        nrx = mid.tile([1, BL], F32, name="nrx")
        nc.scalar.activation(nrx, n2x, AF.Sqrt)
        nc.vector.tensor_scalar_max(nrx, nrx, 1e-12)
        rnx = mid.tile([1, BL], F32, name="rnx")
        nc.vector.reciprocal(rnx, nrx)

        # cross dots + block-diag extraction -> raw scores [BL, K]
        cr = pmid.tile([BL, M], F32, name="cr", tag="pm")
        nc.tensor.matmul(cr, lhsT=fx, rhs=fp, start=True, stop=True)
        crm = mid.tile([BL, M], F32, name="crm")
        nc.vector.tensor_tensor(out=crm, in0=cr, in1=bd, op=ALU.mult)
        sraw = mid.tile([BL, K], F32, name="sraw")
        nc.vector.reduce_sum(out=sraw, in_=crm.rearrange("p (b k) -> p k b", k=K),
                             axis=mybir.AxisListType.X)

        # rnx -> column [BL, 1] ; rnp -> [BL, K]
        rnxt_ps = pmid.tile([BL, 1], F32, name="rnxt_ps", tag="pm")
        nc.tensor.matmul(rnxt_ps, lhsT=rnx, rhs=one11, start=True, stop=True)
        rnpt_ps = pmid.tile([BL, K], F32, name="rnpt_ps", tag="pm")
        rnp_v = rnp.rearrange("o (b k) -> o k b", k=K)
        for k in range(K):
            nc.tensor.matmul(rnpt_ps[:, k:k + 1], lhsT=rnp_v[:, k, :],
                             rhs=one11, start=True, stop=True)
        s1 = mid.tile([BL, K], F32, name="s1")
        nc.vector.tensor_scalar_mul(s1, sraw, rnxt_ps)
        sc_t = mid.tile([BL, K], F32, name="sc_t")
        nc.vector.tensor_tensor(out=sc_t, in0=s1, in1=rnpt_ps, op=ALU.mult)

        # softmax (sharpness) + switch + top2
        mx = mid.tile([BL, 1], F32, name="mx")
        nc.vector.reduce_max(out=mx, in_=sc_t, axis=mybir.AxisListType.X)
        sw = mid.tile([BL, 1], F32, name="sw")
        nc.vector.tensor_scalar_max(sw, mx, 0.0)
        bmx = mid.tile([BL, 1], F32, name="bmx")
        nc.vector.tensor_scalar_mul(bmx, mx, -SHARP)
        ex = mid.tile([BL, K], F32, name="ex")
        nc.scalar.activation(ex, sc_t, AF.Exp, bias=bmx, scale=SHARP)
        zs = mid.tile([BL, 1], F32, name="zs")
        nc.vector.reduce_sum(out=zs, in_=ex, axis=mybir.AxisListType.X)
        rz = mid.tile([BL, 1], F32, name="rz")
        nc.vector.reciprocal(rz, zs)
        wgt = mid.tile([BL, K], F32, name="wgt")
        nc.vector.tensor_scalar_mul(wgt, ex, rz)

        mask1 = mid.tile([BL, K], F32, name="mask1")
        nc.vector.tensor_scalar(out=mask1, in0=sc_t, scalar1=mx, scalar2=None,
                                op0=ALU.is_equal)
        mi1 = mid.tile([BL, K], F32, name="mi1")
        nc.vector.tensor_tensor(out=mi1, in0=mask1, in1=iotak, op=ALU.mult)
        idx1 = mid.tile([BL, 1], F32, name="idx1")
        nc.vector.reduce_max(out=idx1, in_=mi1, axis=mybir.AxisListType.X)
        w1v = mid.tile([BL, 1], F32, name="w1v")
        nc.vector.reduce_max(out=w1v, in_=wgt, axis=mybir.AxisListType.X)

        m2 = mid.tile([BL, K], F32, name="m2")
        nc.vector.tensor_scalar_mul(m2, mask1, 4.0)
        smk = mid.tile([BL, K], F32, name="smk")
        nc.vector.tensor_tensor(out=smk, in0=sc_t, in1=m2, op=ALU.subtract)
        mx2 = mid.tile([BL, 1], F32, name="mx2")
        nc.vector.reduce_max(out=mx2, in_=smk, axis=mybir.AxisListType.X)
        mask2 = mid.tile([BL, K], F32, name="mask2")
        nc.vector.tensor_scalar(out=mask2, in0=smk, scalar1=mx2, scalar2=None,
                                op0=ALU.is_equal)
        mi2 = mid.tile([BL, K], F32, name="mi2")
        nc.vector.tensor_tensor(out=mi2, in0=mask2, in1=iotak, op=ALU.mult)
        idx2 = mid.tile([BL, 1], F32, name="idx2")
        nc.vector.reduce_max(out=idx2, in_=mi2, axis=mybir.AxisListType.X)
        wm2 = mid.tile([BL, K], F32, name="wm2")
        nc.vector.tensor_tensor(out=wm2, in0=wgt, in1=mask2, op=ALU.mult)
        w2v = mid.tile([BL, 1], F32, name="w2v")
        nc.vector.reduce_max(out=w2v, in_=wm2, axis=mybir.AxisListType.X)

        # ---- debug dump tile ----
        dbgt = mid.tile([128, 64], F32, name="dbgt")
        nc.vector.memset(dbgt, 0.0)
        nc.vector.tensor_copy(dbgt[0:BL, 0:K], sc_t)
        nc.vector.tensor_copy(dbgt[0:BL, 8:16], wgt)
        nc.vector.tensor_copy(dbgt[0:BL, 16:17], mx)
        nc.vector.tensor_copy(dbgt[0:BL, 17:18], sw)
        nc.vector.tensor_copy(dbgt[0:BL, 18:19], w1v)
        nc.vector.tensor_copy(dbgt[0:BL, 20:21], idx1)
        nc.vector.tensor_copy(dbgt[0:BL, 24:32], sraw)
        nc.vector.tensor_copy(dbgt[0:BL, 32:40], rnpt_ps)
        nc.vector.tensor_copy(dbgt[0:BL, 40:41], rnxt_ps)

        # if w2 underflowed to 0, gather candidate-1 row instead (stay valid)
        mz = mid.tile([BL, 1], F32, name="mz")
        nc.vector.tensor_scalar(out=mz, in0=w2v, scalar1=0.0, scalar2=None,
                                op0=ALU.is_equal)
        dif = mid.tile([BL, 1], F32, name="dif")
        nc.vector.tensor_tensor(out=dif, in0=idx1, in1=idx2, op=ALU.subtract)
        mzd = mid.tile([BL, 1], F32, name="mzd")
        nc.vector.tensor_tensor(out=mzd, in0=mz, in1=dif, op=ALU.mult)
        idx2c = mid.tile([BL, 1], F32, name="idx2c")
        nc.vector.tensor_tensor(out=idx2c, in0=idx2, in1=mzd, op=ALU.add)

        # scalars for phase B
        w1s = mid.tile([BL, 1], F32, name="w1s")
        nc.vector.tensor_tensor(out=w1s, in0=w1v, in1=sw, op=ALU.mult)
        w2s = mid.tile([BL, 1], F32, name="w2s")
        nc.vector.tensor_tensor(out=w2s, in0=w2v, in1=sw, op=ALU.mult)
        oms = mid.tile([BL, 1], F32, name="oms")
        nc.vector.tensor_scalar(out=oms, in0=sw, scalar1=-1.0, scalar2=1.0,
                                op0=ALU.mult, op1=ALU.add)
        rb1 = mid.tile([BL, 1], F32, name="rb1")
        nc.vector.tensor_tensor(out=rb1, in0=idx1, in1=iotab8, op=ALU.add)
        rb2 = mid.tile([BL, 1], F32, name="rb2")
        nc.vector.tensor_tensor(out=rb2, in0=idx2c, in1=iotab8, op=ALU.add)

        nc.vector.tensor_copy(dbgt[0:BL, 19:20], w2v)
        nc.vector.tensor_copy(dbgt[0:BL, 21:22], idx2c)
        nc.vector.tensor_copy(dbgt[0:BL, 22:23], rb1)
        nc.vector.tensor_copy(dbgt[0:BL, 23:24], rb2)

        # stage -> transpose -> row vector [1, 6*BL]
        stag = mid.tile([BL, 6], F32, name="stag")
        nc.vector.tensor_copy(stag[:, 0:1], w1s)
        nc.vector.tensor_copy(stag[:, 1:2], w2s)
        nc.vector.tensor_copy(stag[:, 2:3], oms)
        nc.vector.tensor_copy(stag[:, 3:4], sw)
        nc.vector.tensor_copy(stag[:, 4:5], rb1)
        nc.vector.tensor_copy(stag[:, 5:6], rb2)
        stagt_ps = pmid.tile([1, 6 * BL], F32, name="stagt_ps", tag="pm")
        for r in range(6):
            nc.tensor.transpose(stagt_ps[:, r * BL:(r + 1) * BL],
                                stag[:, r:r + 1], ident[:BL, :BL])
        srow = mid.tile([1, 6 * BL], F32, name="srow")
        nc.vector.tensor_copy(srow, stagt_ps)

        # broadcast tiles [C, BL] via outer products (ones col x row)
        bc_ps = pmid.tile([C, 6 * BL], F32, name="bc_ps", tag="pm")
        nc.tensor.matmul(bc_ps, lhsT=ones_row, rhs=srow, start=True, stop=True)
        bcall = mid.tile([C, 6 * BL], F32, name="bcall")
        nc.vector.tensor_copy(bcall, bc_ps)
        oms_bc = bcall[:, 2 * BL:3 * BL]

        # w12s stacked on 128 partitions: [w1s*sw ; w2s*sw] columns per b
        w12_ps = pmid.tile([128, BL], F32, name="w12_ps", tag="pm")
        nc.tensor.matmul(w12_ps[0:C, :], lhsT=ones_row, rhs=srow[:, 0:BL],
                         start=True, stop=True)
        nc.tensor.matmul(w12_ps[C:128, :], lhsT=ones_row, rhs=srow[:, BL:2 * BL],
                         start=True, stop=True)
        w12_bc = mid.tile([128, BL], F32, name="w12_bc")
        nc.vector.tensor_copy(w12_bc, w12_ps)

        # sbf[c, b] = bf[c] * sw[b]
        sbf_ps = pmid.tile([C, BL], F32, name="sbf_ps", tag="pm")
        nc.tensor.matmul(sbf_ps, lhsT=bf_row, rhs=srow[:, 3 * BL:4 * BL],
                         start=True, stop=True)
        sbf = mid.tile([C, BL], F32, name="sbf")
        nc.vector.tensor_copy(sbf, sbf_ps)

        # gather indices (int32), one per partition per b:
        # idx[c + 64*cand, b] = 64*(8*b + k_cand) + c
        idx_ps = pmid.tile([128, BL], F32, name="idx_ps", tag="pm")
        nc.tensor.matmul(idx_ps[0:C, :], lhsT=ones_row,
                         rhs=srow[:, 4 * BL:5 * BL], start=True, stop=True)
        nc.tensor.matmul(idx_ps[C:128, :], lhsT=ones_row,
                         rhs=srow[:, 5 * BL:6 * BL], start=True, stop=True)
        idxf12 = mid.tile([128, BL], F32, name="idxf12")
        nc.vector.tensor_scalar(out=idxf12, in0=idx_ps, scalar1=float(C),
                                scalar2=iota_c2, op0=ALU.mult, op1=ALU.add)
        idxi12 = mid.tile([128, BL], DT.int32, name="idxi12")
        nc.vector.tensor_copy(idxi12, idxf12)

        pmid_cm.__exit__(None, None, None)
        pacc_cm.__exit__(None, None, None)

        # =========================== Phase B ===========================
        ps_flat = ps.rearrange("m (c s) -> (m c) s", s=S)   # [M*C, S]
        xs_v = xs.rearrange("b (c s) -> c b s", s=S)        # [C, BL, S]
        out_v = out_d.rearrange("b (c s) -> c b s", s=S)
        with tc.tile_pool(name="pg", bufs=4) as pgp, \
             tc.tile_pool(name="pxb", bufs=2) as pxb, \
             tc.tile_pool(name="pob", bufs=2) as pob, \
             tc.tile_pool(name="pab", bufs=3) as pab, \
             tc.tile_pool(name="patt", bufs=2, space="PSUM") as patt:
            for blk in range(NBLK):
                b0 = blk * NB
                xt = pxb.tile([C, NB, S], F32, name="xt", tag="xt")
                nc.sync.dma_start(xt, xs_v[:, b0:b0 + NB, :])
                ob = pob.tile([C, NB, S], F32, name="ob", tag="ob")
                if blk == 0:
                    nc.vector.tensor_copy(dbgt[0:128, 48:52], idxf12[:, 0:4])
                    nc.vector.tensor_copy(dbgt[0:C, 56:60], w12_bc[0:C, 0:4])
                    nc.vector.tensor_copy(dbgt[0:C, 60:64], sbf[:, 0:4])
                for j in range(NB):
                    b = b0 + j
                    pgs = pgp.tile([128, S], F32, name="pgs", tag="pgs")
                    nc.gpsimd.indirect_dma_start(
                        out=pgs, out_offset=None, in_=ps_flat,
                        in_offset=IndirectOffsetOnAxis(
                            ap=idxi12[:, b:b + 1], axis=0))
                    if blk == 0 and j == 0:
                        nc.vector.tensor_copy(dbgt[:, 44:48], pgs[:, 0:4])
                    abt = pab.tile([128, C], F32, name="abt", tag="abt")
                    nc.vector.tensor_scalar_mul(abt, atp2, w12_bc[:, b:b + 1])
                    att = patt.tile([C, S], F32, name="att", tag="att")
                    nc.tensor.matmul(att, lhsT=abt, rhs=pgs,
                                     start=True, stop=True)
                    o1 = pab.tile([C, S], F32, name="o1", tag="o1")
                    nc.scalar.activation(o1, att, AF.Identity,
                                         bias=sbf[:, b:b + 1], scale=1.0)
                    xsc = pab.tile([C, S], F32, name="xsc", tag="xsc")
                    nc.vector.tensor_scalar_mul(xsc, xt[:, j, :],
                                                oms_bc[:, b:b + 1])
                    nc.vector.tensor_tensor(out=ob[:, j, :], in0=o1, in1=xsc,
                                            op=ALU.add)
                nc.sync.dma_start(out_v[:, b0:b0 + NB, :], ob)
            nc.sync.dma_start(dbg_d, dbgt)


_NC_CACHE = {}


def _get_nc():
    if "nc" not in _NC_CACHE:
        _NC_CACHE["nc"] = build_nc()
    return _NC_CACHE["nc"]


def prepare_in_maps(x, p, theta_w, theta_b, phi_w, phi_b, g_w, g_b, o_w, o_b):
    x = np.asarray(x, dtype=np.float32)
    p = np.asarray(p, dtype=np.float32)
    theta_w = [np.asarray(w, np.float32) for w in theta_w]
    theta_b = [np.asarray(b, np.float32) for b in theta_b]
    phi_w = [np.asarray(w, np.float32) for w in phi_w]
    phi_b = [np.asarray(b, np.float32) for b in phi_b]
    g_w = np.asarray(g_w, np.float32)
    g_b = np.asarray(g_b, np.float32)
    o_w = np.asarray(o_w, np.float32)
    o_b = np.asarray(o_b, np.float32)

    # fused 1x1 convs: att = A @ wsum + bfuse,  A = o_w@g_w
    A = (o_w @ g_w).astype(np.float32)
    bfuse = (o_w @ g_b + o_b).astype(np.float32)

    consts = np.zeros((128, 16), np.float32)
    for i, bvec in enumerate(theta_b[:4]):
        consts[:len(bvec), i] = bvec
    for i, bvec in enumerate(phi_b[:4]):
        consts[:len(bvec), i + 4] = bvec
    consts[:C, 8] = np.arange(C)
    consts[C:2 * C, 8] = np.arange(C)
    consts[:, 9] = 1.0
    consts[:BL, 10] = 8.0 * np.arange(BL)

    crow = np.zeros((1, 128), np.float32)
    crow[0, 0:C] = bfuse
    crow[0, C:C + BL] = 1.0

    bdm = np.zeros((BL, M), np.float32)
    for b in range(BL):
        bdm[b, b * K:(b + 1) * K] = 1.0
    iotak = np.tile(np.arange(K, dtype=np.float32), (BL, 1))
    atp2 = np.vstack([A.T, A.T]).astype(np.float32)
    ident = np.eye(128, dtype=np.float32)

    shared = {
        "w1f": np.ascontiguousarray(phi_w[0]),
        "w1t": np.ascontiguousarray(theta_w[0]),
        "w2f": np.ascontiguousarray(phi_w[1]),
        "w3f": np.ascontiguousarray(phi_w[2]),
        "w4f": np.ascontiguousarray(phi_w[3]),
        "w2t": np.ascontiguousarray(theta_w[1]),
        "w3t": np.ascontiguousarray(theta_w[2]),
        "w4t": np.ascontiguousarray(theta_w[3]),
        "ident": ident, "consts": consts, "crow": crow,
        "bdmask": bdm, "iotak": iotak, "atp2": atp2,
    }
    xf = x.reshape(B, NIN)
    pf = p.reshape(B, K, NIN)
    in_maps = []
    for c in range(NCORES):
        sl = slice(c * BL, (c + 1) * BL)
        in_maps.append({
            "xs": np.ascontiguousarray(xf[sl]),
            "ps": np.ascontiguousarray(pf[sl].reshape(M, NIN)),
            **shared,
        })
    return in_maps


def kernel(**inputs):
    in_maps = prepare_in_maps(**inputs)
    nc = _get_nc()
    kw = {}
    if TRACE:
        kw = dict(trace=True, tmpdir=TRACE_DIR)
    res = run_bass_kernel_spmd(nc, in_maps, core_ids=list(range(NCORES)), **kw)
    kernel.last_results = res
    outs = [res.results[c]["out"] for c in range(NCORES)]
    full = np.concatenate(outs, axis=0)          # [B, NIN]
    return full.reshape(B, C, E, E, E).astype(np.float32)


if __name__ == "__main__":
    nc = build_nc()
    print("built ok:", len(nc.m.functions[0].instructions) if hasattr(nc.m.functions[0], 'instructions') else "n/a")


# revision 17
# speedup vs baseline: 1.0891x; 1.0891x over previous
"""Trainium2 Bass kernel for nn_AttentionBlock (retrieval_knn).

Data-parallel over batch across 8 cores. Per core (64 batches):
  Phase A: stream p (64MB) + x (8MB) + W1s (32MB) over the contraction dim
           (k=32768). PE-transpose 128x128 tiles of p/x to get k on
           partitions, then accumulate H1.T = W1.T @ p.T in PSUM
           (features-on-partitions layout).
  Mid:     MLP layers 2-4 (feature-major, no transposes), L2 norms via
           ones-matmul partition reduction, cross-dot scores, sharp softmax,
           top-2 candidate selection (softmax at sharpness 65536 is ~one-hot;
           residual weight mass < 1e-7 is dropped).
  Phase B: indirect-DMA gather of only the top-2 candidate patches in
           channel-major layout, then one matmul per batch folds the softmax
           weighting + fused g/o 1x1 conv (A = o_w@g_w) + gate: the gathered
           top-2 patches are stacked on 128 partitions and multiplied by
           [w1*sw*A.T ; w2*sw*A.T]. Gated blend with x finishes on DVE/ACT.
"""

import sys

import numpy as np

sys.path.insert(0, "/opt/trn_rl_repo")

import concourse.bass as bass
import concourse.bacc as bacc
import concourse.mybir as mybir
import concourse.tile as tile
from concourse.bass import IndirectOffsetOnAxis
from concourse.bass_utils import run_bass_kernel_spmd

AF = mybir.ActivationFunctionType
ALU = mybir.AluOpType
DT = mybir.dt

NCORES = 8
B, K, C, E = 512, 8, 64, 8
S = E * E * E            # 512
NIN = C * S              # 32768
CF, HID = 32, 128
BL = B // NCORES         # 64 batches per core
M = BL * K               # 512 p-rows per core
KC = 128                 # contraction chunk (partition dim)
NKC = NIN // KC          # 256 chunks
KBLK = 16                # chunks per DMA K-block
NKB = NKC // KBLK        # 16 K-blocks
SHARP = float(CF * S * 4)  # 65536
NB = 8                   # phase-B batch block size
NBLK = BL // NB          # 8 blocks

F32 = DT.float32
TRACE = False
TRACE_DIR = "/root/problem/trace_out"


def _leaky(nc, pool, h_psum, bias_ap, p, m, tagpfx):
    """a = leakyrelu(h + bias, 0.01); h in PSUM [p, m], bias [p, 1]."""
    y = pool.tile([p, m], F32, name=f"{tagpfx}_y", tag=f"{tagpfx}_y")
    nc.scalar.activation(y, h_psum, AF.Identity, bias=bias_ap, scale=1.0)
    t = pool.tile([p, m], F32, name=f"{tagpfx}_t", tag=f"{tagpfx}_t")
    nc.vector.tensor_scalar_mul(t, y, 0.01)
    a = pool.tile([p, m], F32, name=f"{tagpfx}_a", tag=f"{tagpfx}_a")
    nc.vector.tensor_tensor(out=a, in0=y, in1=t, op=ALU.max)
    return a


def build_nc():
    nc = bacc.Bacc("TRN2", target_bir_lowering=False, debug=False,
                   num_devices=NCORES)

    # ---- DRAM I/O ----
    xs = nc.dram_tensor("xs", [BL, NIN], F32, kind="ExternalInput").ap()
    ps = nc.dram_tensor("ps", [M, NIN], F32, kind="ExternalInput").ap()
    ps16 = nc.dram_tensor("ps16", [M, NIN], DT.float16,
                          kind="ExternalInput").ap()
    xs16 = nc.dram_tensor("xs16", [BL, NIN], DT.float16,
                          kind="ExternalInput").ap()
    w1f = nc.dram_tensor("w1f16", [KC, NKC * HID], DT.float16,
                         kind="ExternalInput").ap()
    w1t = nc.dram_tensor("w1t16", [KC, NKC * HID], DT.float16,
                         kind="ExternalInput").ap()
    w2f = nc.dram_tensor("w2f", [HID, HID], F32, kind="ExternalInput").ap()
    w3f = nc.dram_tensor("w3f", [HID, HID], F32, kind="ExternalInput").ap()
    w4f = nc.dram_tensor("w4f", [HID, CF], F32, kind="ExternalInput").ap()
    w2t = nc.dram_tensor("w2t", [HID, HID], F32, kind="ExternalInput").ap()
    w3t = nc.dram_tensor("w3t", [HID, HID], F32, kind="ExternalInput").ap()
    w4t = nc.dram_tensor("w4t", [HID, CF], F32, kind="ExternalInput").ap()
    ident_d = nc.dram_tensor("ident", [128, 128], F32, kind="ExternalInput").ap()
    # consts columns: 0 b1t | 1 b2t | 2 b3t | 3 b4t(32) | 4 b1f | 5 b2f | 6 b3f
    # | 7 b4f(32) | 8 iota_c(64) | 9 ones | 10 8*iota_b(64) | 11..15 zero
    consts_d = nc.dram_tensor("consts", [128, 16], F32, kind="ExternalInput").ap()
    # crow row vector: cols 0:64 bf (= o_w@g_b + o_b), 64:128 ones
    crow_d = nc.dram_tensor("crow", [1, 128], F32, kind="ExternalInput").ap()
    bd_d = nc.dram_tensor("bdmask", [BL, M], F32, kind="ExternalInput").ap()
    iotak_d = nc.dram_tensor("iotak", [BL, K], F32, kind="ExternalInput").ap()
    atp2_d = nc.dram_tensor("atp2", [128, C], F32, kind="ExternalInput").ap()
    out_d = nc.dram_tensor("out", [BL, NIN], F32, kind="ExternalOutput").ap()
    dbg_d = nc.dram_tensor("dbg", [128, 64], F32, kind="ExternalOutput").ap()

    with tile.TileContext(nc) as tc:
        _build(nc, tc, xs, ps, ps16, xs16, w1f, w1t, w2f, w3f, w4f, w2t, w3t, w4t,
               ident_d, consts_d, crow_d, bd_d, iotak_d, atp2_d, out_d, dbg_d)
    nc.compile()
    return nc


def _build(nc, tc, xs, ps, ps16, xs16, w1f, w1t, w2f, w3f, w4f, w2t, w3t, w4t,
           ident_d, consts_d, crow_d, bd_d, iotak_d, atp2_d, out_d, dbg_d):
    from contextlib import ExitStack
    ctx = ExitStack()
    with ctx:
        consts = ctx.enter_context(tc.tile_pool(name="consts", bufs=1))
        # ---- resident constants ----
        ident = consts.tile([128, 128], F32, name="identsb")
        nc.sync.dma_start(ident, ident_d)
        cst = consts.tile([128, 16], F32, name="cstsb")
        nc.sync.dma_start(cst, consts_d)
        crow = consts.tile([1, 128], F32, name="crowsb")
        nc.sync.dma_start(crow, crow_d)
        bd = consts.tile([BL, M], F32, name="bdsb")
        nc.sync.dma_start(bd, bd_d)
        iotak = consts.tile([BL, K], F32, name="iotaksb")
        nc.sync.dma_start(iotak, iotak_d)
        atp2 = consts.tile([128, C], F32, name="atp2sb")
        nc.sync.dma_start(atp2, atp2_d)
        atp2r = consts.tile([128, C], DT.float32r, name="atp2r")
        nc.vector.tensor_copy(atp2r, atp2)
        ident16 = consts.tile([128, 128], DT.float16, name="ident16")
        nc.vector.tensor_copy(ident16, ident)
        w2fs = consts.tile([HID, HID], F32, name="w2fs")
        nc.sync.dma_start(w2fs, w2f)
        w3fs = consts.tile([HID, HID], F32, name="w3fs")
        nc.sync.dma_start(w3fs, w3f)
        w4fs = consts.tile([HID, CF], F32, name="w4fs")
        nc.sync.dma_start(w4fs, w4f)
        w2ts = consts.tile([HID, HID], F32, name="w2ts")
        nc.sync.dma_start(w2ts, w2t)
        w3ts = consts.tile([HID, HID], F32, name="w3ts")
        nc.sync.dma_start(w3ts, w3t)
        w4ts = consts.tile([HID, CF], F32, name="w4ts")
        nc.sync.dma_start(w4ts, w4t)

        b1t, b2t, b3t = (cst[:, i:i + 1] for i in range(3))
        b4t = cst[:CF, 3:4]
        b1f, b2f, b3f = (cst[:, i:i + 1] for i in range(4, 7))
        b4f = cst[:CF, 7:8]
        iota_c = cst[:C, 8:9]
        iota_c2 = cst[:, 8:9]
        ones32 = cst[:CF, 9:10]
        one11 = cst[:1, 9:10]
        iotab8 = cst[:BL, 10:11]
        bf_row = crow[:, 0:C]
        ones_row = crow[:, C:C + BL]

        # persistent accumulators for H1 (whole phase A)
        pacc_cm = tc.tile_pool(name="pacc", bufs=1, space="PSUM")
        pacc = pacc_cm.__enter__()
        h1p = pacc.tile([128, M], F32, name="h1p")
        h1x = pacc.tile([128, BL], F32, name="h1x")

        # =========================== Phase A ===========================
        with tc.tile_pool(name="pin", bufs=2) as pin, \
             tc.tile_pool(name="prhs", bufs=6) as prhs, \
             tc.tile_pool(name="ptp", bufs=3, space="PSUM") as ptp, \
             tc.tile_pool(name="ptx", bufs=2, space="PSUM") as ptx:
            ps_v = ps16.rearrange("(mb mp) n -> mp mb n", mp=128)    # [128,4,NIN]
            for kb in range(NKB):
                k0 = kb * KBLK * KC
                p_nat = []
                for mb in range(4):
                    t = pin.tile([128, KBLK * KC], DT.float16,
                                 name=f"p_nat{mb}", tag=f"p_nat{mb}")
                    nc.sync.dma_start(t, ps_v[:, mb, k0:k0 + KBLK * KC])
                    p_nat.append(t)
                x_nat = pin.tile([BL, KBLK * KC], DT.float16, name="x_nat",
                                 tag="x_nat")
                nc.sync.dma_start(x_nat, xs16[:, k0:k0 + KBLK * KC])
                w1f_b = pin.tile([128, KBLK * HID], DT.float16, name="w1f_b",
                                 tag="w1f_b")
                nc.sync.dma_start(w1f_b, w1f[:, kb * KBLK * HID:
                                             (kb + 1) * KBLK * HID])
                w1t_b = pin.tile([128, KBLK * HID], DT.float16, name="w1t_b",
                                 tag="w1t_b")
                nc.sync.dma_start(w1t_b, w1t[:, kb * KBLK * HID:
                                             (kb + 1) * KBLK * HID])
                for sc in range(KBLK):
                    g = kb * KBLK + sc
                    off = sc * KC
                    tp = ptp.tile([128, M], DT.float16, name="tp", tag="tp")
                    for mb in range(4):
                        nc.tensor.transpose(
                            tp[:, mb * 128:(mb + 1) * 128],
                            p_nat[mb][:, off:off + KC], ident16)
                    tx = ptx.tile([128, BL], DT.float16, name="tx", tag="tx")
                    nc.tensor.transpose(tx, x_nat[:, off:off + KC],
                                        ident16[:BL, :BL])
                    rhsp = prhs.tile([128, M], DT.float16, name="rhsp",
                                     tag="rhsp")
                    nc.vector.tensor_copy(rhsp, tp)
                    rhsx = prhs.tile([128, BL], DT.float16, name="rhsx",
                                     tag="rhsx")
                    nc.scalar.copy(rhsx, tx)
                    nc.tensor.matmul(h1p, lhsT=w1f_b[:, off:off + HID],
                                     rhs=rhsp,
                                     start=(g == 0), stop=(g == NKC - 1))
                    nc.tensor.matmul(h1x, lhsT=w1t_b[:, off:off + HID],
                                     rhs=rhsx,
                                     start=(g == 0), stop=(g == NKC - 1))

        # =========================== Mid phase =========================
        mid = ctx.enter_context(tc.tile_pool(name="mid", bufs=1))
        pmid_cm = tc.tile_pool(name="pmid", bufs=5, space="PSUM")
        pmid = pmid_cm.__enter__()

        # MLP layers 2..4, p path (feature-major: [feat, m])
        a1p = _leaky(nc, mid, h1p, b1f, 128, M, "a1p")
        h2p = pmid.tile([128, M], F32, name="h2p", tag="pm")
        nc.tensor.matmul(h2p, lhsT=w2fs, rhs=a1p, start=True, stop=True)
        a2p = _leaky(nc, mid, h2p, b2f, 128, M, "a2p")
        h3p = pmid.tile([128, M], F32, name="h3p", tag="pm")
        nc.tensor.matmul(h3p, lhsT=w3fs, rhs=a2p, start=True, stop=True)
        a3p = _leaky(nc, mid, h3p, b3f, 128, M, "a3p")
        h4p = pmid.tile([CF, M], F32, name="h4p", tag="pm")
        nc.tensor.matmul(h4p, lhsT=w4fs, rhs=a3p, start=True, stop=True)
        # x path
        a1x = _leaky(nc, mid, h1x, b1t, 128, BL, "a1x")
        h2x = pmid.tile([128, BL], F32, name="h2x", tag="pm")
        nc.tensor.matmul(h2x, lhsT=w2ts, rhs=a1x, start=True, stop=True)
        a2x = _leaky(nc, mid, h2x, b2t, 128, BL, "a2x")
        h3x = pmid.tile([128, BL], F32, name="h3x", tag="pm")
        nc.tensor.matmul(h3x, lhsT=w3ts, rhs=a2x, start=True, stop=True)
        a3x = _leaky(nc, mid, h3x, b3t, 128, BL, "a3x")
        h4x = pmid.tile([CF, BL], F32, name="h4x", tag="pm")
        nc.tensor.matmul(h4x, lhsT=w4ts, rhs=a3x, start=True, stop=True)

        # features + squares (bias folded in)
        fp = mid.tile([CF, M], F32, name="fp")
        nc.scalar.activation(fp, h4p, AF.Identity, bias=b4f, scale=1.0)
        sqp = mid.tile([CF, M], F32, name="sqp")
        nc.scalar.activation(sqp, h4p, AF.Square, bias=b4f, scale=1.0)
        fx = mid.tile([CF, BL], F32, name="fx")
        nc.scalar.activation(fx, h4x, AF.Identity, bias=b4t, scale=1.0)
        sqx = mid.tile([CF, BL], F32, name="sqx")
        nc.scalar.activation(sqx, h4x, AF.Square, bias=b4t, scale=1.0)

        # L2 norms: partition-reduce via ones matmul
        n2p = pmid.tile([1, M], F32, name="n2p", tag="pm")
        nc.tensor.matmul(n2p, lhsT=ones32, rhs=sqp, start=True, stop=True)
        n2x = pmid.tile([1, BL], F32, name="n2x", tag="pm")
        nc.tensor.matmul(n2x, lhsT=ones32, rhs=sqx, start=True, stop=True)
        nrp = mid.tile([1, M], F32, name="nrp")
        nc.scalar.activation(nrp, n2p, AF.Sqrt)
        nc.vector.tensor_scalar_max(nrp, nrp, 1e-12)
        rnp = mid.tile([1, M], F32, name="rnp")
        nc.vector.reciprocal(rnp, nrp)
        nrx = mid.tile([1, BL], F32, name="nrx")
        nc.scalar.activation(nrx, n2x, AF.Sqrt)
        nc.vector.tensor_scalar_max(nrx, nrx, 1e-12)
        rnx = mid.tile([1, BL], F32, name="rnx")
        nc.vector.reciprocal(rnx, nrx)

        # cross dots + block-diag extraction -> raw scores [BL, K]
        cr = pmid.tile([BL, M], F32, name="cr", tag="pm")
        nc.tensor.matmul(cr, lhsT=fx, rhs=fp, start=True, stop=True)
        crm = mid.tile([BL, M], F32, name="crm")
        nc.vector.tensor_tensor(out=crm, in0=cr, in1=bd, op=ALU.mult)
        sraw = mid.tile([BL, K], F32, name="sraw")
        nc.vector.reduce_sum(out=sraw, in_=crm.rearrange("p (b k) -> p k b", k=K),
                             axis=mybir.AxisListType.X)

        # rnx -> column [BL, 1] ; rnp -> [BL, K]
        rnxt_ps = pmid.tile([BL, 1], F32, name="rnxt_ps", tag="pm")
        nc.tensor.matmul(rnxt_ps, lhsT=rnx, rhs=one11, start=True, stop=True)
        rnpt_ps = pmid.tile([BL, K], F32, name="rnpt_ps", tag="pm")
        rnp_v = rnp.rearrange("o (b k) -> o k b", k=K)
        for k in range(K):
            nc.tensor.matmul(rnpt_ps[:, k:k + 1], lhsT=rnp_v[:, k, :],
                             rhs=one11, start=True, stop=True)
        s1 = mid.tile([BL, K], F32, name="s1")
        nc.vector.tensor_scalar_mul(s1, sraw, rnxt_ps)
        sc_t = mid.tile([BL, K], F32, name="sc_t")
        nc.vector.tensor_tensor(out=sc_t, in0=s1, in1=rnpt_ps, op=ALU.mult)

        # softmax (sharpness) + switch + top2
        mx = mid.tile([BL, 1], F32, name="mx")
        nc.vector.reduce_max(out=mx, in_=sc_t, axis=mybir.AxisListType.X)
        sw = mid.tile([BL, 1], F32, name="sw")
        nc.vector.tensor_scalar_max(sw, mx, 0.0)
        bmx = mid.tile([BL, 1], F32, name="bmx")
        nc.vector.tensor_scalar_mul(bmx, mx, -SHARP)
        ex = mid.tile([BL, K], F32, name="ex")
        nc.scalar.activation(ex, sc_t, AF.Exp, bias=bmx, scale=SHARP)
        zs = mid.tile([BL, 1], F32, name="zs")
        nc.vector.reduce_sum(out=zs, in_=ex, axis=mybir.AxisListType.X)
        rz = mid.tile([BL, 1], F32, name="rz")
        nc.vector.reciprocal(rz, zs)
        wgt = mid.tile([BL, K], F32, name="wgt")
        nc.vector.tensor_scalar_mul(wgt, ex, rz)

        mask1 = mid.tile([BL, K], F32, name="mask1")
        nc.vector.tensor_scalar(out=mask1, in0=sc_t, scalar1=mx, scalar2=None,
                                op0=ALU.is_equal)
        mi1 = mid.tile([BL, K], F32, name="mi1")
        nc.vector.tensor_tensor(out=mi1, in0=mask1, in1=iotak, op=ALU.mult)
        idx1 = mid.tile([BL, 1], F32, name="idx1")
        nc.vector.reduce_max(out=idx1, in_=mi1, axis=mybir.AxisListType.X)
        w1v = mid.tile([BL, 1], F32, name="w1v")
        nc.vector.reduce_max(out=w1v, in_=wgt, axis=mybir.AxisListType.X)

        m2 = mid.tile([BL, K], F32, name="m2")
        nc.vector.tensor_scalar_mul(m2, mask1, 4.0)
        smk = mid.tile([BL, K], F32, name="smk")
        nc.vector.tensor_tensor(out=smk, in0=sc_t, in1=m2, op=ALU.subtract)
        mx2 = mid.tile([BL, 1], F32, name="mx2")
        nc.vector.reduce_max(out=mx2, in_=smk, axis=mybir.AxisListType.X)
        mask2 = mid.tile([BL, K], F32, name="mask2")
        nc.vector.tensor_scalar(out=mask2, in0=smk, scalar1=mx2, scalar2=None,
                                op0=ALU.is_equal)
        mi2 = mid.tile([BL, K], F32, name="mi2")
        nc.vector.tensor_tensor(out=mi2, in0=mask2, in1=iotak, op=ALU.mult)
        idx2 = mid.tile([BL, 1], F32, name="idx2")
        nc.vector.reduce_max(out=idx2, in_=mi2, axis=mybir.AxisListType.X)
        wm2 = mid.tile([BL, K], F32, name="wm2")
        nc.vector.tensor_tensor(out=wm2, in0=wgt, in1=mask2, op=ALU.mult)
        w2v = mid.tile([BL, 1], F32, name="w2v")
        nc.vector.reduce_max(out=w2v, in_=wm2, axis=mybir.AxisListType.X)

        # ---- debug dump tile ----
        dbgt = mid.tile([128, 64], F32, name="dbgt")
        nc.vector.memset(dbgt, 0.0)
        nc.vector.tensor_copy(dbgt[0:BL, 0:K], sc_t)
        nc.vector.tensor_copy(dbgt[0:BL, 8:16], wgt)
        nc.vector.tensor_copy(dbgt[0:BL, 16:17], mx)
        nc.vector.tensor_copy(dbgt[0:BL, 17:18], sw)
        nc.vector.tensor_copy(dbgt[0:BL, 18:19], w1v)
        nc.vector.tensor_copy(dbgt[0:BL, 20:21], idx1)
        nc.vector.tensor_copy(dbgt[0:BL, 24:32], sraw)
        nc.vector.tensor_copy(dbgt[0:BL, 32:40], rnpt_ps)
        nc.vector.tensor_copy(dbgt[0:BL, 40:41], rnxt_ps)

        # if w2 underflowed to 0, gather candidate-1 row instead (stay valid)
        mz = mid.tile([BL, 1], F32, name="mz")
        nc.vector.tensor_scalar(out=mz, in0=w2v, scalar1=0.0, scalar2=None,
                                op0=ALU.is_equal)
        dif = mid.tile([BL, 1], F32, name="dif")
        nc.vector.tensor_tensor(out=dif, in0=idx1, in1=idx2, op=ALU.subtract)
        mzd = mid.tile([BL, 1], F32, name="mzd")
        nc.vector.tensor_tensor(out=mzd, in0=mz, in1=dif, op=ALU.mult)
        idx2c = mid.tile([BL, 1], F32, name="idx2c")
        nc.vector.tensor_tensor(out=idx2c, in0=idx2, in1=mzd, op=ALU.add)

        # scalars for phase B
        w1s = mid.tile([BL, 1], F32, name="w1s")
        nc.vector.tensor_tensor(out=w1s, in0=w1v, in1=sw, op=ALU.mult)
        w2s = mid.tile([BL, 1], F32, name="w2s")
        nc.vector.tensor_tensor(out=w2s, in0=w2v, in1=sw, op=ALU.mult)
        oms = mid.tile([BL, 1], F32, name="oms")
        nc.vector.tensor_scalar(out=oms, in0=sw, scalar1=-1.0, scalar2=1.0,
                                op0=ALU.mult, op1=ALU.add)
        rb1 = mid.tile([BL, 1], F32, name="rb1")
        nc.vector.tensor_tensor(out=rb1, in0=idx1, in1=iotab8, op=ALU.add)
        rb2 = mid.tile([BL, 1], F32, name="rb2")
        nc.vector.tensor_tensor(out=rb2, in0=idx2c, in1=iotab8, op=ALU.add)

        nc.vector.tensor_copy(dbgt[0:BL, 19:20], w2v)
        nc.vector.tensor_copy(dbgt[0:BL, 21:22], idx2c)
        nc.vector.tensor_copy(dbgt[0:BL, 22:23], rb1)
        nc.vector.tensor_copy(dbgt[0:BL, 23:24], rb2)

        # stage -> transpose -> row vector [1, 6*BL]
        stag = mid.tile([BL, 6], F32, name="stag")
        nc.vector.tensor_copy(stag[:, 0:1], w1s)
        nc.vector.tensor_copy(stag[:, 1:2], w2s)
        nc.vector.tensor_copy(stag[:, 2:3], oms)
        nc.vector.tensor_copy(stag[:, 3:4], sw)
        nc.vector.tensor_copy(stag[:, 4:5], rb1)
        nc.vector.tensor_copy(stag[:, 5:6], rb2)
        stagt_ps = pmid.tile([1, 6 * BL], F32, name="stagt_ps", tag="pm")
        for r in range(6):
            nc.tensor.transpose(stagt_ps[:, r * BL:(r + 1) * BL],
                                stag[:, r:r + 1], ident[:BL, :BL])
        srow = mid.tile([1, 6 * BL], F32, name="srow")
        nc.vector.tensor_copy(srow, stagt_ps)

        # broadcast tiles [C, BL] via outer products (ones col x row)
        bc_ps = pmid.tile([C, 6 * BL], F32, name="bc_ps", tag="pm")
        nc.tensor.matmul(bc_ps, lhsT=ones_row, rhs=srow, start=True, stop=True)
        bcall = mid.tile([C, 6 * BL], F32, name="bcall")
        nc.vector.tensor_copy(bcall, bc_ps)
        oms_bc = bcall[:, 2 * BL:3 * BL]

        # w12s stacked on 128 partitions: [w1s*sw ; w2s*sw] columns per b
        w12_ps = pmid.tile([128, BL], F32, name="w12_ps", tag="pm")
        nc.tensor.matmul(w12_ps[0:C, :], lhsT=ones_row, rhs=srow[:, 0:BL],
                         start=True, stop=True)
        nc.tensor.matmul(w12_ps[C:128, :], lhsT=ones_row, rhs=srow[:, BL:2 * BL],
                         start=True, stop=True)
        w12_bc = mid.tile([128, BL], F32, name="w12_bc")
        nc.vector.tensor_copy(w12_bc, w12_ps)

        # sbf[c, b] = bf[c] * sw[b]
        sbf_ps = pmid.tile([C, BL], F32, name="sbf_ps", tag="pm")
        nc.tensor.matmul(sbf_ps, lhsT=bf_row, rhs=srow[:, 3 * BL:4 * BL],
                         start=True, stop=True)
        sbf = mid.tile([C, BL], F32, name="sbf")
        nc.vector.tensor_copy(sbf, sbf_ps)

        # gather indices (int32), one per partition per b:
        # idx[c + 64*cand, b] = 64*(8*b + k_cand) + c
        idx_ps = pmid.tile([128, BL], F32, name="idx_ps", tag="pm")
        nc.tensor.matmul(idx_ps[0:C, :], lhsT=ones_row,
                         rhs=srow[:, 4 * BL:5 * BL], start=True, stop=True)
        nc.tensor.matmul(idx_ps[C:128, :], lhsT=ones_row,
                         rhs=srow[:, 5 * BL:6 * BL], start=True, stop=True)
        idxf12 = mid.tile([128, BL], F32, name="idxf12")
        nc.vector.tensor_scalar(out=idxf12, in0=idx_ps, scalar1=float(C),
                                scalar2=iota_c2, op0=ALU.mult, op1=ALU.add)
        idxi12 = mid.tile([128, BL], DT.int32, name="idxi12")
        nc.vector.tensor_copy(idxi12, idxf12)

        pmid_cm.__exit__(None, None, None)
        pacc_cm.__exit__(None, None, None)

        # =========================== Phase B ===========================
        ps_flat = ps.rearrange("m (c s) -> (m c) s", s=S)   # [M*C, S]
        xs_v = xs.rearrange("b (c s) -> c b s", s=S)        # [C, BL, S]
        out_v = out_d.rearrange("b (c s) -> c b s", s=S)
        with tc.tile_pool(name="pg", bufs=8) as pgp, \
             tc.tile_pool(name="pxb", bufs=1) as pxb, \
             tc.tile_pool(name="pob", bufs=2) as pob, \
             tc.tile_pool(name="pab", bufs=4) as pab, \
             tc.tile_pool(name="patt", bufs=4, space="PSUM") as patt:
            XPF = 5
            xts = {}

            def _xt_load(blk):
                b0 = blk * NB
                t = pxb.tile([C, NB, S], F32, name=f"xt{blk}",
                             tag=f"xt{blk % XPF}")
                nc.sync.dma_start(t, xs_v[:, b0:b0 + NB, :])
                xts[blk] = t

            for blk in range(min(XPF, NBLK)):
                _xt_load(blk)
            for blk in range(NBLK):
                if blk + XPF < NBLK:
                    _xt_load(blk + XPF)
                b0 = blk * NB
                xt = xts.pop(blk)
                ob = pob.tile([C, NB, S], F32, name="ob", tag="ob")
                if blk == 0:
                    nc.vector.tensor_copy(dbgt[0:128, 48:52], idxf12[:, 0:4])
                    nc.vector.tensor_copy(dbgt[0:C, 56:60], w12_bc[0:C, 0:4])
                    nc.vector.tensor_copy(dbgt[0:C, 60:64], sbf[:, 0:4])
                for j in range(NB):
                    b = b0 + j
                    pgs = pgp.tile([128, S], DT.float32r, name="pgs",
                                   tag="pgs")
                    nc.gpsimd.indirect_dma_start(
                        out=pgs, out_offset=None, in_=ps_flat,
                        in_offset=IndirectOffsetOnAxis(
                            ap=idxi12[:, b:b + 1], axis=0))
                    if blk == 0 and j == 0:
                        nc.vector.tensor_copy(dbgt[:, 44:48],
                                              pgs[:, 0:4].bitcast(F32))
                    abt = pab.tile([128, C], DT.float32r, name="abt",
                                   tag="abt")
                    nc.vector.tensor_scalar_mul(abt, atp2r, w12_bc[:, b:b + 1])
                    att = patt.tile([C, S], F32, name="att", tag="att")
                    nc.tensor.matmul(att, lhsT=abt, rhs=pgs,
                                     start=True, stop=True)
                    o1 = pab.tile([C, S], F32, name="o1", tag="o1")
                    nc.scalar.activation(o1, att, AF.Identity,
                                         bias=sbf[:, b:b + 1], scale=1.0)
                    xsc = pab.tile([C, S], F32, name="xsc", tag="xsc")
                    nc.vector.tensor_scalar_mul(xsc, xt[:, j, :],
                                                oms_bc[:, b:b + 1])
                    nc.vector.tensor_tensor(out=ob[:, j, :], in0=o1, in1=xsc,
                                            op=ALU.add)
                nc.sync.dma_start(out_v[:, b0:b0 + NB, :], ob)
            nc.sync.dma_start(dbg_d, dbgt)


_NC_CACHE = {}


def _get_nc():
    if "nc" not in _NC_CACHE:
        _NC_CACHE["nc"] = build_nc()
    return _NC_CACHE["nc"]


def prepare_in_maps(x, p, theta_w, theta_b, phi_w, phi_b, g_w, g_b, o_w, o_b):
    x = np.asarray(x, dtype=np.float32)
    p = np.asarray(p, dtype=np.float32)
    theta_w = [np.asarray(w, np.float32) for w in theta_w]
    theta_b = [np.asarray(b, np.float32) for b in theta_b]
    phi_w = [np.asarray(w, np.float32) for w in phi_w]
    phi_b = [np.asarray(b, np.float32) for b in phi_b]
    g_w = np.asarray(g_w, np.float32)
    g_b = np.asarray(g_b, np.float32)
    o_w = np.asarray(o_w, np.float32)
    o_b = np.asarray(o_b, np.float32)

    # fused 1x1 convs: att = A @ wsum + bfuse,  A = o_w@g_w
    A = (o_w @ g_w).astype(np.float32)
    bfuse = (o_w @ g_b + o_b).astype(np.float32)

    consts = np.zeros((128, 16), np.float32)
    for i, bvec in enumerate(theta_b[:4]):
        consts[:len(bvec), i] = bvec
    for i, bvec in enumerate(phi_b[:4]):
        consts[:len(bvec), i + 4] = bvec
    consts[:C, 8] = np.arange(C)
    consts[C:2 * C, 8] = np.arange(C)
    consts[:, 9] = 1.0
    consts[:BL, 10] = 8.0 * np.arange(BL)

    crow = np.zeros((1, 128), np.float32)
    crow[0, 0:C] = bfuse
    crow[0, C:C + BL] = 1.0

    bdm = np.zeros((BL, M), np.float32)
    for b in range(BL):
        bdm[b, b * K:(b + 1) * K] = 1.0
    iotak = np.tile(np.arange(K, dtype=np.float32), (BL, 1))
    atp2 = np.vstack([A.T, A.T]).astype(np.float32)
    ident = np.eye(128, dtype=np.float32)

    w1f16 = np.ascontiguousarray(
        phi_w[0].reshape(NKC, KC, HID).transpose(1, 0, 2)
        .reshape(KC, NKC * HID).astype(np.float16))
    w1t16 = np.ascontiguousarray(
        theta_w[0].reshape(NKC, KC, HID).transpose(1, 0, 2)
        .reshape(KC, NKC * HID).astype(np.float16))
    shared = {
        "w1f16": w1f16,
        "w1t16": w1t16,
        "w2f": np.ascontiguousarray(phi_w[1]),
        "w3f": np.ascontiguousarray(phi_w[2]),
        "w4f": np.ascontiguousarray(phi_w[3]),
        "w2t": np.ascontiguousarray(theta_w[1]),
        "w3t": np.ascontiguousarray(theta_w[2]),
        "w4t": np.ascontiguousarray(theta_w[3]),
        "ident": ident, "consts": consts, "crow": crow,
        "bdmask": bdm, "iotak": iotak, "atp2": atp2,
    }
    xf = x.reshape(B, NIN)
    pf = p.reshape(B, K, NIN)
    in_maps = []
    for c in range(NCORES):
        sl = slice(c * BL, (c + 1) * BL)
        psc = np.ascontiguousarray(pf[sl].reshape(M, NIN))
        xsc_ = np.ascontiguousarray(xf[sl])
        in_maps.append({
            "xs": xsc_,
            "ps": psc,
            "xs16": xsc_.astype(np.float16),
            "ps16": psc.astype(np.float16),
            **shared,
        })
    return in_maps


def kernel(**inputs):
    in_maps = prepare_in_maps(**inputs)
    nc = _get_nc()
    kw = {}
    if TRACE:
        kw = dict(trace=True, tmpdir=TRACE_DIR)
    res = run_bass_kernel_spmd(nc, in_maps, core_ids=list(range(NCORES)), **kw)
    kernel.last_results = res
    outs = [res.results[c]["out"] for c in range(NCORES)]
    full = np.concatenate(outs, axis=0)          # [B, NIN]
    return full.reshape(B, C, E, E, E).astype(np.float32)


if __name__ == "__main__":
    nc = build_nc()
    print("built ok:", len(nc.m.functions[0].instructions) if hasattr(nc.m.functions[0], 'instructions') else "n/a")
